# revision 22
# baseline (speedup 1.0000x reference)
"""nn_Cvx_ShortestPathNet — TRN2 Bass kernel, 8-core pure data parallelism.

Math (Dykstra alternating projections, c folded into G via a bias lane):
    G = A' pinv(AA') A  (projector),  c = b' pinv(AA') A
    Ghat[768,768]: Ghat[:760,:760] = G, Ghat[760,:760] = -c, Ghat[760,760] = 1
    negw lane 760 := 1 (via the b2 bias constant), so t lane 760 stays 1 and
    t@Ghat == t@G - c on real lanes.
    t_1 = negw = -MLP(d);  t_{k+1} = max(negw, t_k @ Ghat)   (pure tensor max)
    y = max(negw - t_K @ Ghat, 0) = max(ps, negw) - ps

On-chip layout transposed ([768, B_local], 6x128 partition tiles), B_local =
32 per core. Per iteration: 36 fp16 matmuls + 3 DVE tensor_tensor max ops.

PSUM bank plan: `start=True` clears has_written for the WHOLE bank, so two
accumulation groups may not interleave within a bank. Each pair tile is
[128, 1024] fp32 = TWO banks: group 2p writes cols 480:512 (end of bank 0),
group 2p+1 writes cols 512:544 (start of bank 1). Groups get private banks
(interleave freely) while the pair's DVE max reads one contiguous [128,64].
3 pair tiles (6 banks) + MLP [128,512] x2 (2 banks) = all 8 banks.

The matmul order staggers pair closes (21, 25, 37 of 37 slots) so each DVE
result is ready before the next iteration's matmuls consume it (steady state
R_consume = [2,6,18], R_war = [4,8,20] slot positions; zero PE stalls).

Batch 256 sharded 32 rows per core; Ghat, MLP weights replicated.
"""

import json
import numpy as np

import concourse.bass as bass
import concourse.mybir as mybir
import concourse.tile as tile
from concourse.bass_utils import run_bass_kernel_spmd

F32 = mybir.dt.float32
F16 = mybir.dt.float16
AT = mybir.AluOpType
AF = mybir.ActivationFunctionType

JT = 6          # 768/128 edge-dim tiles
BL = 32         # batch rows per core
HT = 5          # 640/128 hidden tiles
K_ITERS = 100
N_CORES = 8
N2 = 760
NP = JT * 128
PCOL = 480      # pair tile: group 2p at cols 480:512, group 2p+1 at 512:544

# DVE op partition: groups covered by each tensor_tensor max op. Singles own
# one PSUM bank; pairs own two adjacent banks (group at each bank edge).
PARTITION = [(0,), (1,), (2,), (3,), (4, 5)]
# Staggered (j,k) order (EDF, lam=[20,20,20,20,22]): group closes 23/25/27/
# 29/35/41, op closes 23/25/27/29/41.
MM_ORDER = [
    (0, 1), (0, 0), (0, 2), (1, 2), (0, 3), (1, 3), (1, 1), (1, 0), (2, 3),
    (2, 2), (2, 1), (2, 0), (3, 3), (3, 2), (3, 1), (3, 0), (0, 4), (0, 5),
    (1, 4), (1, 5), (2, 4), (2, 5), (3, 4), (3, 5), (4, 4), (4, 5), (4, 3),
    (4, 2), (4, 1), (4, 0), (5, 4), (5, 5), (5, 3), (5, 2), (5, 1), (5, 0),
]
_first = {}
_last = {}
for _pos, (_j, _k) in enumerate(MM_ORDER):
    _first.setdefault(_j, _pos)
    _last[_j] = _pos
_OP_OF_GROUP = {}
for _oi, _S in enumerate(PARTITION):
    for _g in _S:
        _OP_OF_GROUP[_g] = _oi

# ---------------------------------------------------------------------------
# This container's walrus build rejects instructions carrying more than one
# sync-wait. Split any multi-wait instruction at the BIR-JSON level: insert
# same-engine NoOps before it, each carrying one of the extra waits (waits
# are sem-ge, so order is irrelevant).
_orig_to_json_bytes = bass.Bass.to_json_bytes
_ctr = [0]


def _order_waits(engine: str, waits: list) -> list:
    """NoOps take the waits that are (almost surely) already satisfied --
    same-engine sems and DMA arrivals -- so the instruction keeps the
    latest-firing cross-engine wait and pays no NoOp decode after it."""
    def prio(w):
        nm = w.get("ant_name", "")
        if nm.startswith(engine + "_"):
            return 0
        if nm.startswith("DMA"):
            return 1
        if nm.startswith("PE_"):
            return 3
        return 2
    return sorted(waits, key=prio)


def _split_waits_json(raw: bytes) -> bytes:
    j = json.loads(raw)
    changed = False
    for fn in j.get("functions", []):
        for bb in fn.get("blocks", []):
            out = []
            for inst in bb.get("instructions", []):
                si = inst.get("sync_info") or {}
                waits = si.get("on_wait") or []
                if len(waits) > 1:
                    changed = True
                    waits = _order_waits(inst.get("engine", ""), waits)
                    for w in waits[:-1]:
                        _ctr[0] += 1
                        out.append({
                            "debug": inst.get("debug", 0),
                            "engine": inst["engine"],
                            "ins": [], "outs": [],
                            "name": f"I-waitsplit-{_ctr[0]}",
                            "opcode": "NoOp",
                            "sync_info": {"on_wait": [w], "on_update": []},
                        })
                    si["on_wait"] = waits[-1:]
                out.append(inst)
            bb["instructions"] = out
    return json.dumps(j).encode() if changed else raw


def _patched_to_json_bytes(self, *a, **k):
    return _split_waits_json(_orig_to_json_bytes(self, *a, **k))


bass.Bass.to_json_bytes = _patched_to_json_bytes


def _build(k_iters=K_ITERS):
    nc = bass.Bass("TRN2", target_bir_lowering=False, debug=False,
                   num_devices=N_CORES)

    g_mat = nc.dram_tensor("g_mat", [128, JT * JT * 128], F16, kind="ExternalInput").ap()
    w2t = nc.dram_tensor("w2t", [128, HT * JT * 128], F16, kind="ExternalInput").ap()
    w1 = nc.dram_tensor("w1", [64, HT * 128], F16, kind="ExternalInput").ap()
    dt_in = nc.dram_tensor("dt_in", [64, BL], F16, kind="ExternalInput").ap()
    b1c = nc.dram_tensor("b1c", [128, HT], F32, kind="ExternalInput").ap()
    nb2c = nc.dram_tensor("nb2c", [128, JT], F32, kind="ExternalInput").ap()
    y_out = nc.dram_tensor("y_out", [128, JT * BL], F16, kind="ExternalOutput").ap()

    NOP = len(PARTITION)

    def nw_base(oi):
        return sum(len(S) for S in PARTITION[:oi]) * BL

    with tile.TileContext(nc) as tc:
        with (
            tc.tile_pool(name="const", bufs=1) as cpool,
            tc.tile_pool(name="state", bufs=2) as spool,
            tc.tile_pool(name="psum", bufs=1, space="PSUM") as ppool,
            tc.tile_pool(name="psum2", bufs=2, space="PSUM") as p2pool,
        ):
            # --- input DMAs over the 3 DMA queues (sync/scalar HWDGE, SWDGE)
            dT_sb = cpool.tile([64, BL], F16)
            nc.sync.dma_start(out=dT_sb[:], in_=dt_in[:])
            w1_sb = cpool.tile([64, HT * 128], F16)
            nc.sync.dma_start(out=w1_sb[:], in_=w1[:])
            b1c_sb = cpool.tile([128, HT], F32)
            nc.sync.dma_start(out=b1c_sb[:], in_=b1c[:])
            nb2c_sb = cpool.tile([128, JT], F32)
            nc.sync.dma_start(out=nb2c_sb[:], in_=nb2c[:])
            # W2 j-major: chunk j holds blocks (j*HT + k2); iteration 1 can
            # start once chunk 0 + G k0 land.
            w2_sb = cpool.tile([128, HT * JT * 128], F16)
            wq = [nc.sync, nc.scalar]
            for j in range(JT):
                sl = slice(j * HT * 128, (j + 1) * HT * 128)
                wq[j % 2].dma_start(out=w2_sb[:, sl], in_=w2t[:, sl])
            G_sb = cpool.tile([128, JT * JT * 128], F16)
            gq = [nc.gpsimd, nc.gpsimd, nc.gpsimd, nc.gpsimd, nc.sync, nc.scalar]
            for k in range(JT):
                sl = slice(k * JT * 128, (k + 1) * JT * 128)
                gq[k].dma_start(out=G_sb[:, sl], in_=g_mat[:, sl])

            # pair PSUM tiles (bufs=1): two banks each, group at each bank
            # edge so the pair's DVE read is one contiguous [128,64].
            # (start=True clears has_written bank-wide -> private banks.)
            # The warm-up and MLP reuse these banks before iterations begin.
            # ops 0,1 (consumed right at iteration start) get per-iteration
            # double-buffered banks from p2pool; the rest use fixed banks
            p2_ops = (0, 1)
            ps_fixed = {oi: ppool.tile([128, 512 * len(S)], F32, tag=f"ps{oi}",
                                       name=f"psp{oi}")
                        for oi, S in enumerate(PARTITION) if oi not in p2_ops}
            fixed_list = [ps_fixed[oi] for oi in sorted(ps_fixed)]

            # --- HAM warm-up: dummy matmuls keep the PE busy through the
            # DMA phase so the clock gate reaches K=8/8 before the real work
            for i in range(80):
                nc.tensor.matmul(out=fixed_list[i % len(fixed_list)][:32, :BL],
                                 lhsT=dT_sb[:, :BL],
                                 rhs=dT_sb[:], start=True, stop=True)

            # --- MLP: h = leaky_relu(d@W1 + b1), negw = -(h@W2 + b2) -------
            h16 = cpool.tile([128, HT * BL], F16)
            for m in range(HT):
                ph = fixed_list[m % len(fixed_list)]
                nc.tensor.matmul(out=ph[:, :BL],
                                 lhsT=w1_sb[:, m * 128:(m + 1) * 128],
                                 rhs=dT_sb[:], start=True, stop=True)
                pre = spool.tile([128, BL], F32, tag="pre", name=f"pre{m}")
                nc.scalar.activation(out=pre[:, :], in_=ph[:, :BL],
                                     func=AF.Identity,
                                     bias=b1c_sb[:, m:m + 1], scale=1.0)
                # leaky relu = max(x, 0.1x) on DVE
                nc.vector.scalar_tensor_tensor(
                    out=h16[:, m * BL:(m + 1) * BL], in0=pre[:],
                    scalar=0.1, in1=pre[:], op0=AT.mult, op1=AT.max)
            # negw per DVE-op tiles so iteration 1 starts as chunks land
            negw = [cpool.tile([128, len(PARTITION[oi]) * BL], F32,
                               name=f"negw{oi}") for oi in range(NOP)]
            negw16 = [cpool.tile([128, len(PARTITION[oi]) * BL], F16,
                                 name=f"negw16_{oi}") for oi in range(NOP)]
            for j in range(JT):
                pw = fixed_list[j % len(fixed_list)]
                for k2 in range(HT):
                    nc.tensor.matmul(
                        out=pw[:, :BL],
                        lhsT=w2_sb[:, (j * HT + k2) * 128:(j * HT + k2 + 1) * 128],
                        rhs=h16[:, k2 * BL:(k2 + 1) * BL],
                        start=(k2 == 0), stop=(k2 == HT - 1))
                # negw = -(ps + b2); lane 760 gets +1 via nb2c[120,5] = 1
                oi = _OP_OF_GROUP[j]
                lc = (j - PARTITION[oi][0]) * BL
                nc.scalar.activation(out=negw[oi][:, lc:lc + BL],
                                     in_=pw[:, :BL], func=AF.Identity,
                                     bias=nb2c_sb[:, j:j + 1], scale=-1.0)

            # t_1 = negw (fp16); lane 760 == 1 already via negw
            t_cur = []
            for oi in range(NOP):
                nc.vector.tensor_copy(out=negw16[oi][:], in_=negw[oi][:])
                t0 = spool.tile([128, len(PARTITION[oi]) * BL], F16,
                                tag=f"t{oi}", name=f"t0_{oi}")
                nc.vector.tensor_copy(out=t0[:], in_=negw16[oi][:])
                t_cur.append(t0)

            # --- Dykstra iterations ---------------------------------------
            for it in range(k_iters):
                last_iter = it == k_iters - 1
                # singles get per-iteration double-buffered banks (no WAR on
                # the next iteration's start=True); pairs reuse fixed banks
                ps_it = {}
                for oi in range(NOP):
                    if oi in ps_fixed:
                        ps_it[oi] = ps_fixed[oi]
                    else:
                        ps_it[oi] = p2pool.tile([128, 512], F32, tag=f"psg{oi}",
                                                name=f"psg{oi}_{it}")

                def out_ap(j):
                    oi = _OP_OF_GROUP[j]
                    idx = PARTITION[oi].index(j)
                    return ps_it[oi][:, PCOL + idx * BL:PCOL + (idx + 1) * BL]

                if not last_iter:
                    t_nxt = [spool.tile([128, len(S) * BL], F16, tag=f"t{oi}",
                                        name=f"t{it + 1}_{oi}")
                             for oi, S in enumerate(PARTITION)]
                else:
                    tl = cpool.tile([128, JT * BL], F32)
                    y_sb = cpool.tile([128, JT * BL], F16)
                left = {j: 6 for j in range(JT)}
                for pos, (j, k) in enumerate(MM_ORDER):
                    ko = _OP_OF_GROUP[k]
                    kc = PARTITION[ko].index(k) * BL
                    nc.tensor.matmul(
                        out=out_ap(j),
                        lhsT=G_sb[:, (k * JT + j) * 128:(k * JT + j + 1) * 128],
                        rhs=t_cur[ko][:, kc:kc + BL],
                        start=(pos == _first[j]), stop=(pos == _last[j]))
                    left[j] -= 1
                    oi = _OP_OF_GROUP[j]
                    if all(left[g] == 0 for g in PARTITION[oi]):
                        for g in PARTITION[oi]:
                            left[g] = -1  # fire once
                        n = len(PARTITION[oi]) * BL
                        pss = ps_it[oi][:, PCOL:PCOL + n]
                        if not last_iter:
                            nc.vector.tensor_tensor(out=t_nxt[oi][:], in0=pss,
                                                    in1=negw16[oi][:],
                                                    op=AT.max)
                        else:
                            gl = slice(nw_base(oi), nw_base(oi) + n)
                            nc.vector.tensor_tensor(out=tl[:, gl], in0=pss,
                                                    in1=negw[oi][:], op=AT.max)
                            nc.vector.tensor_tensor(out=y_sb[:, gl],
                                                    in0=tl[:, gl], in1=pss,
                                                    op=AT.subtract)
                if not last_iter:
                    t_cur = t_nxt
                else:
                    nc.sync.dma_start(out=y_out[:], in_=y_sb[:])
    return nc


def _host_prepare(d, W1, b1, W2, b2, A, b_eq):
    A64 = A.astype(np.float64)
    M = np.linalg.pinv(A64 @ A64.T)
    G = A64.T @ M @ A64
    c = (b_eq.astype(np.float64) @ M) @ A64

    n2 = A.shape[1]
    Ghat = np.zeros((NP, NP), np.float64)
    Ghat[:n2, :n2] = G
    Ghat[n2, :n2] = -c          # bias lane row
    Ghat[n2, n2] = 1.0

    g_sb = (Ghat.reshape(JT, 128, JT, 128).transpose(1, 0, 2, 3)
            .reshape(128, JT * JT * 128)).astype(np.float16)

    HID = W1.shape[1]
    W2_pad = np.zeros((HID, NP), np.float64)
    W2_pad[:, :n2] = W2.astype(np.float64)
    w2_sb = (W2_pad.reshape(HT, 128, JT, 128).transpose(1, 2, 0, 3)
             .reshape(128, JT * HT * 128)).astype(np.float16)
    b1c = b1.reshape(HT, 128).T.astype(np.float32).copy()
    b2_pad = np.zeros(NP, np.float32)
    b2_pad[:n2] = b2
    nb2c = (-b2_pad).reshape(JT, 128).T.astype(np.float32).copy()
    nb2c[n2 - 5 * 128, 5] = 1.0   # lane 760 -> partition 120, block 5

    shared = {"g_mat": g_sb, "w2t": w2_sb, "w1": W1.astype(np.float16),
              "b1c": b1c, "nb2c": nb2c}
    B = d.shape[0]
    bl = B // N_CORES
    in_maps = []
    for i in range(N_CORES):
        dT = d[i * bl:(i + 1) * bl, :].T.astype(np.float16).copy()
        in_maps.append({**shared, "dt_in": dT})
    return in_maps


_nc_cache = {}


def kernel(d, W1, b1, W2, b2, A, b_eq):
    d = np.asarray(d, np.float32)
    W1 = np.asarray(W1, np.float32)
    b1 = np.asarray(b1, np.float32)
    W2 = np.asarray(W2, np.float32)
    b2 = np.asarray(b2, np.float32)
    A = np.asarray(A, np.float32)
    b_eq = np.asarray(b_eq, np.float32)

    if "nc" not in _nc_cache:
        _nc_cache["nc"] = _build()
    nc = _nc_cache["nc"]

    in_maps = _host_prepare(d, W1, b1, W2, b2, A, b_eq)
    res = run_bass_kernel_spmd(nc, in_maps, list(range(N_CORES)))

    outs = []
    for r in res.results:
        y = (r["y_out"].reshape(128, JT, BL).transpose(2, 1, 0)
             .reshape(BL, JT * 128))
        outs.append(y[:, :N2])
    return np.concatenate(outs, axis=0).astype(np.float32)


# revision 24
# speedup vs baseline: 1.1953x; 1.1953x over previous
"""nn_Cvx_ShortestPathNet — TRN2 Bass kernel, 8-core pure data parallelism.

Math (Dykstra alternating projections, c folded into G via a bias lane):
    G = A' pinv(AA') A  (projector),  c = b' pinv(AA') A
    Ghat[768,768]: Ghat[:760,:760] = G, Ghat[760,:760] = -c, Ghat[760,760] = 1
    negw lane 760 := 1 (via the b2 bias constant), so t lane 760 stays 1 and
    t@Ghat == t@G - c on real lanes.
    t_1 = negw = -MLP(d);  t_{k+1} = max(negw, t_k @ Ghat)   (pure tensor max)
    y = max(negw - t_K @ Ghat, 0) = max(ps, negw) - ps

On-chip layout transposed ([768, B_local], 6x128 partition tiles), B_local =
32 per core. Per iteration: 36 fp16 matmuls (j-block accumulation groups,
steady pitch ~25ns, LDWEIGHTS/FWL-bound) + 4 DVE tensor_tensor max ops
(PARTITION below), scheduled so each op's result is ready before the next
iteration's matmuls consume it.

PSUM bank plan: `start=True` clears has_written for the WHOLE bank, so two
accumulation groups may not interleave within a bank -> private banks per
group. A pair op's tile is [128,1024] fp32 = TWO banks with the groups at
the bank edges (cols 480:512 | 512:544) so its DVE max reads one contiguous
[128,64]. Ops 0,1 (consumed right at iteration start) double-buffer across
iterations via p2pool (no write-after-read wait on their banks). The HAM
warm-up matmuls and the MLP reuse the fixed banks before iterations start.
2x2 (singles, bufs=2) + 2+2 (pairs) = all 8 banks.

Startup: inputs ride the 3 DMA queues (SP/Act HWDGE + SWDGE) with W2 in
j-major chunks so MLP layer 2 and iteration 1 start as data lands; dummy
warm-up matmuls keep the PE busy through the DMA phase for the HAM clock.

Batch 256 sharded 32 rows per core; Ghat, MLP weights replicated.
"""

import json
import numpy as np

import concourse.bass as bass
import concourse.mybir as mybir
import concourse.tile as tile
from concourse.bass_utils import run_bass_kernel_spmd

F32 = mybir.dt.float32
F16 = mybir.dt.float16
AT = mybir.AluOpType
AF = mybir.ActivationFunctionType

JT = 6          # 768/128 edge-dim tiles
BL = 32         # batch rows per core
HT = 5          # 640/128 hidden tiles
K_ITERS = 100
N_CORES = 8
N2 = 760
NP = JT * 128
PCOL = 480      # pair tile: group 2p at cols 480:512, group 2p+1 at 512:544

# DVE op partition: groups covered by each tensor_tensor max op. Singles own
# one PSUM bank; pairs own two adjacent banks (group at each bank edge).
PARTITION = [(0,), (1,), (2, 3), (4, 5)]
# Staggered (j,k) order (EDF, lam=[18,18,20,20]): group closes 21/23/25/27/
# 33/39, op closes 21/23/27/39.
MM_ORDER = [
    (0, 1), (0, 0), (1, 1), (1, 0), (0, 2), (0, 3), (1, 2), (1, 3), (2, 2),
    (2, 3), (2, 1), (2, 0), (3, 2), (3, 3), (3, 1), (3, 0), (0, 4), (0, 5),
    (1, 4), (1, 5), (2, 4), (2, 5), (3, 4), (3, 5), (4, 4), (4, 5), (4, 2),
    (4, 3), (4, 1), (4, 0), (5, 4), (5, 5), (5, 2), (5, 3), (5, 1), (5, 0),
]
_first = {}
_last = {}
for _pos, (_j, _k) in enumerate(MM_ORDER):
    _first.setdefault(_j, _pos)
    _last[_j] = _pos
_OP_OF_GROUP = {}
for _oi, _S in enumerate(PARTITION):
    for _g in _S:
        _OP_OF_GROUP[_g] = _oi

# ---------------------------------------------------------------------------
# This container's walrus build rejects instructions carrying more than one
# sync-wait. Split any multi-wait instruction at the BIR-JSON level: insert
# same-engine NoOps before it, each carrying one of the extra waits (waits
# are sem-ge, so order is irrelevant).
_orig_to_json_bytes = bass.Bass.to_json_bytes
_ctr = [0]


def _order_waits(engine: str, waits: list) -> list:
    """NoOps take the waits that are (almost surely) already satisfied --
    same-engine sems and DMA arrivals -- so the instruction keeps the
    latest-firing cross-engine wait and pays no NoOp decode after it."""
    def prio(w):
        nm = w.get("ant_name", "")
        if nm.startswith(engine + "_"):
            return 0
        if nm.startswith("DMA"):
            return 1
        if nm.startswith("PE_"):
            return 3
        return 2
    return sorted(waits, key=prio)


def _split_waits_json(raw: bytes) -> bytes:
    j = json.loads(raw)
    changed = False
    for fn in j.get("functions", []):
        for bb in fn.get("blocks", []):
            out = []
            for inst in bb.get("instructions", []):
                si = inst.get("sync_info") or {}
                waits = si.get("on_wait") or []
                if len(waits) > 1:
                    changed = True
                    waits = _order_waits(inst.get("engine", ""), waits)
                    for w in waits[:-1]:
                        _ctr[0] += 1
                        out.append({
                            "debug": inst.get("debug", 0),
                            "engine": inst["engine"],
                            "ins": [], "outs": [],
                            "name": f"I-waitsplit-{_ctr[0]}",
                            "opcode": "NoOp",
                            "sync_info": {"on_wait": [w], "on_update": []},
                        })
                    si["on_wait"] = waits[-1:]
                out.append(inst)
            bb["instructions"] = out
    return json.dumps(j).encode() if changed else raw


def _patched_to_json_bytes(self, *a, **k):
    return _split_waits_json(_orig_to_json_bytes(self, *a, **k))


bass.Bass.to_json_bytes = _patched_to_json_bytes


def _build(k_iters=K_ITERS):
    nc = bass.Bass("TRN2", target_bir_lowering=False, debug=False,
                   num_devices=N_CORES)

    g_mat = nc.dram_tensor("g_mat", [128, JT * JT * 128], F16, kind="ExternalInput").ap()
    w2t = nc.dram_tensor("w2t", [128, HT * JT * 128], F16, kind="ExternalInput").ap()
    w1 = nc.dram_tensor("w1", [64, HT * 128], F16, kind="ExternalInput").ap()
    dt_in = nc.dram_tensor("dt_in", [64, BL], F16, kind="ExternalInput").ap()
    b1c = nc.dram_tensor("b1c", [128, HT], F32, kind="ExternalInput").ap()
    nb2c = nc.dram_tensor("nb2c", [128, JT], F32, kind="ExternalInput").ap()
    y_out = nc.dram_tensor("y_out", [128, JT * BL], F16, kind="ExternalOutput").ap()

    NOP = len(PARTITION)

    def nw_base(oi):
        return sum(len(S) for S in PARTITION[:oi]) * BL

    with tile.TileContext(nc) as tc:
        with (
            tc.tile_pool(name="const", bufs=1) as cpool,
            tc.tile_pool(name="state", bufs=2) as spool,
            tc.tile_pool(name="psum", bufs=1, space="PSUM") as ppool,
            tc.tile_pool(name="psum2", bufs=2, space="PSUM") as p2pool,
        ):
            # --- input DMAs over the 3 DMA queues (sync/scalar HWDGE, SWDGE)
            dT_sb = cpool.tile([64, BL], F16)
            nc.sync.dma_start(out=dT_sb[:], in_=dt_in[:])
            w1_sb = cpool.tile([64, HT * 128], F16)
            nc.sync.dma_start(out=w1_sb[:], in_=w1[:])
            b1c_sb = cpool.tile([128, HT], F32)
            nc.sync.dma_start(out=b1c_sb[:], in_=b1c[:])
            nb2c_sb = cpool.tile([128, JT], F32)
            nc.sync.dma_start(out=nb2c_sb[:], in_=nb2c[:])
            # W2 j-major: chunk j holds blocks (j*HT + k2); iteration 1 can
            # start once chunk 0 + G k0 land.
            w2_sb = cpool.tile([128, HT * JT * 128], F16)
            wq = [nc.sync, nc.scalar]
            for j in range(JT):
                sl = slice(j * HT * 128, (j + 1) * HT * 128)
                wq[j % 2].dma_start(out=w2_sb[:, sl], in_=w2t[:, sl])
            G_sb = cpool.tile([128, JT * JT * 128], F16)
            gq = [nc.gpsimd, nc.gpsimd, nc.gpsimd, nc.gpsimd, nc.sync, nc.scalar]
            for k in range(JT):
                sl = slice(k * JT * 128, (k + 1) * JT * 128)
                gq[k].dma_start(out=G_sb[:, sl], in_=g_mat[:, sl])

            # pair PSUM tiles (bufs=1): two banks each, group at each bank
            # edge so the pair's DVE read is one contiguous [128,64].
            # (start=True clears has_written bank-wide -> private banks.)
            # The warm-up and MLP reuse these banks before iterations begin.
            # ops 0,1 (consumed right at iteration start) get per-iteration
            # double-buffered banks from p2pool; the rest use fixed banks
            p2_ops = (0, 1)
            ps_fixed = {oi: ppool.tile([128, 512 * len(S)], F32, tag=f"ps{oi}",
                                       name=f"psp{oi}")
                        for oi, S in enumerate(PARTITION) if oi not in p2_ops}
            fixed_list = [ps_fixed[oi] for oi in sorted(ps_fixed)]

            # --- HAM warm-up: dummy matmuls keep the PE busy through the
            # DMA phase so the clock gate reaches K=8/8 before the real work
            for i in range(80):
                nc.tensor.matmul(out=fixed_list[i % len(fixed_list)][:32, :BL],
                                 lhsT=dT_sb[:, :BL],
                                 rhs=dT_sb[:], start=True, stop=True)

            # --- MLP: h = leaky_relu(d@W1 + b1), negw = -(h@W2 + b2) -------
            h16 = cpool.tile([128, HT * BL], F16)
            for m in range(HT):
                ph = fixed_list[m % len(fixed_list)]
                nc.tensor.matmul(out=ph[:, :BL],
                                 lhsT=w1_sb[:, m * 128:(m + 1) * 128],
                                 rhs=dT_sb[:], start=True, stop=True)
                pre = spool.tile([128, BL], F32, tag="pre", name=f"pre{m}")
                nc.scalar.activation(out=pre[:, :], in_=ph[:, :BL],
                                     func=AF.Identity,
                                     bias=b1c_sb[:, m:m + 1], scale=1.0)
                # leaky relu = max(x, 0.1x) on DVE
                nc.vector.scalar_tensor_tensor(
                    out=h16[:, m * BL:(m + 1) * BL], in0=pre[:],
                    scalar=0.1, in1=pre[:], op0=AT.mult, op1=AT.max)
            # negw per DVE-op tiles so iteration 1 starts as chunks land
            negw = [cpool.tile([128, len(PARTITION[oi]) * BL], F32,
                               name=f"negw{oi}") for oi in range(NOP)]
            negw16 = [cpool.tile([128, len(PARTITION[oi]) * BL], F16,
                                 name=f"negw16_{oi}") for oi in range(NOP)]
            for j in range(JT):
                pw = fixed_list[j % len(fixed_list)]
                for k2 in range(HT):
                    nc.tensor.matmul(
                        out=pw[:, :BL],
                        lhsT=w2_sb[:, (j * HT + k2) * 128:(j * HT + k2 + 1) * 128],
                        rhs=h16[:, k2 * BL:(k2 + 1) * BL],
                        start=(k2 == 0), stop=(k2 == HT - 1))
                # negw = -(ps + b2); lane 760 gets +1 via nb2c[120,5] = 1
                oi = _OP_OF_GROUP[j]
                lc = (j - PARTITION[oi][0]) * BL
                nc.scalar.activation(out=negw[oi][:, lc:lc + BL],
                                     in_=pw[:, :BL], func=AF.Identity,
                                     bias=nb2c_sb[:, j:j + 1], scale=-1.0)

            # t_1 = negw (fp16); lane 760 == 1 already via negw
            t_cur = []
            for oi in range(NOP):
                nc.vector.tensor_copy(out=negw16[oi][:], in_=negw[oi][:])
                t0 = spool.tile([128, len(PARTITION[oi]) * BL], F16,
                                tag=f"t{oi}", name=f"t0_{oi}")
                nc.vector.tensor_copy(out=t0[:], in_=negw16[oi][:])
                t_cur.append(t0)

            # --- Dykstra iterations ---------------------------------------
            for it in range(k_iters):
                last_iter = it == k_iters - 1
                # singles get per-iteration double-buffered banks (no WAR on
                # the next iteration's start=True); pairs reuse fixed banks
                ps_it = {}
                for oi in range(NOP):
                    if oi in ps_fixed:
                        ps_it[oi] = ps_fixed[oi]
                    else:
                        ps_it[oi] = p2pool.tile([128, 512], F32, tag=f"psg{oi}",
                                                name=f"psg{oi}_{it}")

                def out_ap(j):
                    oi = _OP_OF_GROUP[j]
                    idx = PARTITION[oi].index(j)
                    return ps_it[oi][:, PCOL + idx * BL:PCOL + (idx + 1) * BL]

                if not last_iter:
                    t_nxt = [spool.tile([128, len(S) * BL], F16, tag=f"t{oi}",
                                        name=f"t{it + 1}_{oi}")
                             for oi, S in enumerate(PARTITION)]
                else:
                    tl = cpool.tile([128, JT * BL], F32)
                    y_sb = cpool.tile([128, JT * BL], F16)
                left = {j: 6 for j in range(JT)}
                for pos, (j, k) in enumerate(MM_ORDER):
                    ko = _OP_OF_GROUP[k]
                    kc = PARTITION[ko].index(k) * BL
                    nc.tensor.matmul(
                        out=out_ap(j),
                        lhsT=G_sb[:, (k * JT + j) * 128:(k * JT + j + 1) * 128],
                        rhs=t_cur[ko][:, kc:kc + BL],
                        start=(pos == _first[j]), stop=(pos == _last[j]))
                    left[j] -= 1
                    oi = _OP_OF_GROUP[j]
                    if all(left[g] == 0 for g in PARTITION[oi]):
                        for g in PARTITION[oi]:
                            left[g] = -1  # fire once
                        n = len(PARTITION[oi]) * BL
                        pss = ps_it[oi][:, PCOL:PCOL + n]
                        if not last_iter:
                            nc.vector.tensor_tensor(out=t_nxt[oi][:], in0=pss,
                                                    in1=negw16[oi][:],
                                                    op=AT.max)
                        else:
                            gl = slice(nw_base(oi), nw_base(oi) + n)
                            nc.vector.tensor_tensor(out=tl[:, gl], in0=pss,
                                                    in1=negw[oi][:], op=AT.max)
                            nc.vector.tensor_tensor(out=y_sb[:, gl],
                                                    in0=tl[:, gl], in1=pss,
                                                    op=AT.subtract)
                if not last_iter:
                    t_cur = t_nxt
                else:
                    nc.sync.dma_start(out=y_out[:], in_=y_sb[:])
    return nc


def _host_prepare(d, W1, b1, W2, b2, A, b_eq):
    A64 = A.astype(np.float64)
    M = np.linalg.pinv(A64 @ A64.T)
    G = A64.T @ M @ A64
    c = (b_eq.astype(np.float64) @ M) @ A64

    n2 = A.shape[1]
    Ghat = np.zeros((NP, NP), np.float64)
    Ghat[:n2, :n2] = G
    Ghat[n2, :n2] = -c          # bias lane row
    Ghat[n2, n2] = 1.0

    g_sb = (Ghat.reshape(JT, 128, JT, 128).transpose(1, 0, 2, 3)
            .reshape(128, JT * JT * 128)).astype(np.float16)

    HID = W1.shape[1]
    W2_pad = np.zeros((HID, NP), np.float64)
    W2_pad[:, :n2] = W2.astype(np.float64)
    w2_sb = (W2_pad.reshape(HT, 128, JT, 128).transpose(1, 2, 0, 3)
             .reshape(128, JT * HT * 128)).astype(np.float16)
    b1c = b1.reshape(HT, 128).T.astype(np.float32).copy()
    b2_pad = np.zeros(NP, np.float32)
    b2_pad[:n2] = b2
    nb2c = (-b2_pad).reshape(JT, 128).T.astype(np.float32).copy()
    nb2c[n2 - 5 * 128, 5] = 1.0   # lane 760 -> partition 120, block 5

    shared = {"g_mat": g_sb, "w2t": w2_sb, "w1": W1.astype(np.float16),
              "b1c": b1c, "nb2c": nb2c}
    B = d.shape[0]
    bl = B // N_CORES
    in_maps = []
    for i in range(N_CORES):
        dT = d[i * bl:(i + 1) * bl, :].T.astype(np.float16).copy()
        in_maps.append({**shared, "dt_in": dT})
    return in_maps


_nc_cache = {}


def kernel(d, W1, b1, W2, b2, A, b_eq):
    d = np.asarray(d, np.float32)
    W1 = np.asarray(W1, np.float32)
    b1 = np.asarray(b1, np.float32)
    W2 = np.asarray(W2, np.float32)
    b2 = np.asarray(b2, np.float32)
    A = np.asarray(A, np.float32)
    b_eq = np.asarray(b_eq, np.float32)

    if "nc" not in _nc_cache:
        _nc_cache["nc"] = _build()
    nc = _nc_cache["nc"]

    in_maps = _host_prepare(d, W1, b1, W2, b2, A, b_eq)
    res = run_bass_kernel_spmd(nc, in_maps, list(range(N_CORES)))

    outs = []
    for r in res.results:
        y = (r["y_out"].reshape(128, JT, BL).transpose(2, 1, 0)
             .reshape(BL, JT * 128))
        outs.append(y[:, :N2])
    return np.concatenate(outs, axis=0).astype(np.float32)
